# revision 19
# baseline (speedup 1.0000x reference)
"""KDE HyperGraph Conv kernel for 8 Trainium2 NeuronCores.

Math: the reference builds H[i,j] = [rho_i > rho_j] (+ self loop), so every
downstream quantity depends only on the *ranks* of the KDE densities rho.
With r_i = rank(rho_i) (ascending) and g_i = M-1-r_i (# of larger rhos):

    De_j = g_j + 1,  Dv_i = M - g_i
    A[i,k] = C(min(r_i, r_k)),  C(r) = H_M - H_{M-1-r}   (harmonic numbers)
    y_i = dvs_i * (  sum_{r_k < r_i} C_k dvs_k x_k
                   + C_i * sum_{r_k >= r_i} dvs_k x_k ),  dvs = Dv^-1/2

which turns the [M,M]@[M,M] propagation into two [M,M]@[M,C] matmuls with
the 0/1 comparison matrix L[i,k] = [rho_k < rho_i] plus elementwise work.
C(r) is evaluated with the asymptotic expansion of harmonic numbers.

Distance pass: PE computes d2'[i,j] = x_i.x_j - sq_j/2 into PSUM; the
ScalarEngine applies scale/bias (adding sq_i per-partition) while doing
sqrt / exp, with a free row-sum (accum_out) giving h and rho directly.

Data-parallel over batch: each of the 8 cores handles B/8 = 2 batches.
"""

import os
import sys

for _p in ("/opt/trn_rl_repo",):
    if os.path.isdir(_p) and _p not in sys.path:
        sys.path.append(_p)

import numpy as np

import concourse.bass as bass
import concourse.tile as tile
from concourse import bacc, mybir
from concourse.bass_utils import run_bass_kernel_spmd
from concourse.masks import make_identity

N_CORES = 8
B, M, C = 16, 2048, 128
NB = B // N_CORES          # batches per core
NT = M // 128              # 128-row chunks per batch
NS = M // 512              # 512-wide column slices
EPS = 0.1                  # diagonal-safety shift added inside sqrt
                           # (covers float32r rounding noise on d2 diag)
GAMMA = 0.5772156649015329
H_M = float((1.0 / np.arange(1, M + 1, dtype=np.float64)).sum())

F32 = mybir.dt.float32
BF16 = mybir.dt.bfloat16
F32R = mybir.dt.float32r
AF = mybir.ActivationFunctionType
ALU = mybir.AluOpType
AX = mybir.AxisListType


def _tb(t):
    return slice(t * 128, (t + 1) * 128)


def _sl(j):
    return slice(j * 512, (j + 1) * 512)


def build_kernel():
    nc = bacc.Bacc("TRN2", target_bir_lowering=False, debug=False)

    # Per-core inputs (host pre-permuted, see make_in_maps below):
    #   xT[b, c, i]          = x[b, i, c]            (channels on partitions)
    #   xN[b, p, t*128 + c]  = x[b, t*128 + p, c]    (rows on partitions, chunked)
    #   WT[c, o]             = W[o, c]               (projection, pre-transposed)
    # Output yH[b, o, i] = y[b, i, o]  (transposed; host un-permutes)
    xT = nc.declare_dram_parameter("xT", [NB, 128, M], F32, isOutput=False)
    xN = nc.declare_dram_parameter("xN", [NB, 128, M], F32, isOutput=False)
    WT = nc.declare_dram_parameter("WT", [128, 128], F32, isOutput=False)
    yH = nc.declare_dram_parameter("yH", [NB, 128, M], F32, isOutput=True)

    with tile.TileContext(nc) as tc:
        from contextlib import ExitStack

        with ExitStack() as ctx:
            cp = ctx.enter_context(tc.tile_pool(name="consts", bufs=1))
            pb = ctx.enter_context(tc.tile_pool(name="big", bufs=2))
            psm = ctx.enter_context(tc.tile_pool(name="small", bufs=2))
            pp = ctx.enter_context(
                tc.tile_pool(name="psum", bufs=2, space=bass.MemorySpace.PSUM)
            )

            ones_col = cp.tile([128, 1], F32, tag="ones_col")
            nc.gpsimd.memset(ones_col[:, :], 1.0)
            ones_col_r = cp.tile([128, 2], F32R, tag="ones_col_r")
            nc.vector.tensor_copy(ones_col_r[:, 0:1], ones_col[:, :])
            nc.vector.tensor_copy(ones_col_r[:, 1:2], ones_col[:, :])
            ones_row = cp.tile([1, 128], F32, tag="ones_row")
            nc.gpsimd.memset(ones_row[:, :], 1.0)
            ones_row_r = cp.tile([1, 128], F32R, tag="ones_row_r")
            nc.vector.tensor_copy(ones_row_r[:, :], ones_row[:, :])
            mhalf_f32 = cp.tile([1, 128], F32, tag="mhalf_f32")
            nc.gpsimd.memset(mhalf_f32[:, :], -0.5)
            mhalf_row = cp.tile([1, 128], F32R, tag="mhalf_row")
            nc.vector.tensor_copy(mhalf_row[:, :], mhalf_f32[:, :])
            ident = cp.tile([128, 128], F32, tag="ident")
            make_identity(nc, ident[:, :])
            wt_sb = cp.tile([128, 128], F32, tag="wt")
            nc.sync.dma_start(wt_sb[:, :], WT[:, :])
            wt_r = cp.tile([128, 128], F32R, tag="wt_r")
            nc.vector.tensor_copy(wt_r[:, :], wt_sb[:, :])

            st = [dict() for _ in range(NB)]

            # ---------- phase: prep (loads, squares, sq row + chunk) ----------
            for b in range(NB):
                s = st[b]
                xt = pb.tile([128, M], F32, tag="xt", bufs=1, name=f"xt{b}")
                nc.sync.dma_start(xt[:, :], xT[b])
                xt_r = pb.tile([128, M], F32R, tag="xt_r", name=f"xtr{b}")
                nc.vector.tensor_copy(xt_r[:, :], xt[:, :])
                xn = pb.tile([128, M], F32, tag="xn", name=f"xn{b}")
                nc.sync.dma_start(xn[:, :], xN[b])
                xsq = pb.tile([128, M], F32R, tag="xsq", bufs=1, name=f"xsq{b}")
                nc.scalar.activation(xsq[:, :], xt_r[:, :], AF.Square)
                sq_ps = pp.tile([1, M], F32, tag="big", name=f"sqps{b}")
                for j in range(NS):
                    nc.tensor.matmul(
                        sq_ps[:, _sl(j)], lhsT=ones_col_r[:, 0:1], rhs=xsq[:, _sl(j)]
                    )
                sq_row = psm.tile([1, M], F32R, tag="sq_row", name=f"sqrow{b}")
                nc.scalar.copy(sq_row[0:1, :], sq_ps[:, :])
                # sq in chunk layout [128, NT] via DMA + PE transpose so the
                # ACT bias sees the same PE-computed sq as the PSUM d2'
                s16 = psm.tile([NT, 128], F32, tag="s16", name=f"s16_{b}")
                nc.sync.dma_start(s16[:, :], sq_row[0:1, :].bitcast(F32))
                sqc_ps = pp.tile([128, NT], F32, tag="big", name=f"sqcps{b}")
                nc.tensor.transpose(sqc_ps[:, :], s16[:, :], ident[0:NT, 0:NT])
                sqc = psm.tile([128, NT], F32, tag="sqc", name=f"sqc{b}")
                nc.scalar.copy(sqc[:, :], sqc_ps[:, :])
                bias_s = psm.tile([128, NT], F32, tag="bias_s", name=f"biass{b}")
                nc.vector.tensor_scalar_add(bias_s[:, :], sqc[:, :], EPS)
                s["xt_r"], s["xn"], s["sq_row"], s["sqc"], s["bias_s"] = (
                    xt_r, xn, sq_row, sqc, bias_s,
                )

            def emit_d2(s, t, d2_ps):
                # PSUM d2'[i, j] = x_i.x_j - sq_j/2   (i in chunk t)
                # full d2 = -2*d2' + sq_i (+EPS), applied via ACT scale/bias
                for j in range(NS):
                    nc.tensor.matmul(
                        d2_ps[:, _sl(j)],
                        lhsT=mhalf_row[0:1, :],
                        rhs=s["sq_row"][0:1, _sl(j)],
                        start=True,
                        stop=False,
                    )
                for j in range(NS):
                    nc.tensor.matmul(
                        d2_ps[:, _sl(j)],
                        lhsT=s["xt_r"][:, _tb(t)],
                        rhs=s["xt_r"][:, _sl(j)],
                        start=False,
                        stop=True,
                    )

            # ---------- phase: pass B (sampled dist sums for h; sqrt set) ----
            # h is a mean over 4.2M off-diag distances; a 4-tile (512-row)
            # sample estimates it to ~4e-5 relative, far below the spacing
            # of adjacent rho values, so ranks are unaffected.
            HS = (0, 4, 8, 12)
            for b in range(NB):
                s = st[b]
                hacc = psm.tile([128, len(HS)], F32, tag="hacc", name=f"hacc{b}")
                for hi, t in enumerate(HS):
                    d2_ps = pp.tile([128, M], F32, tag="big", name=f"d2b{b}_{t}")
                    emit_d2(s, t, d2_ps)
                    scr = pb.tile([128, M], BF16, tag="scr", bufs=1, name=f"sb{b}_{t}")
                    nc.scalar.activation(
                        scr[:, :],
                        d2_ps[:, :],
                        AF.Sqrt,
                        scale=-2.0,
                        bias=s["bias_s"][:, t : t + 1],
                        accum_out=hacc[:, hi : hi + 1],
                    )
                s["hacc"] = hacc

            # ---------- phase: h -> f2 = 1/h^2, exp bias -------------------
            for b in range(NB):
                s = st[b]
                hsum = psm.tile([128, 1], F32, tag="hsum", name=f"hsum{b}")
                nc.vector.tensor_reduce(
                    hsum[:, :], s["hacc"][:, :], axis=AX.X, op=ALU.add
                )
                tot_ps = pp.tile([1, 1], F32, tag="big", name=f"tot{b}")
                nc.tensor.matmul(tot_ps[:, :], lhsT=ones_col[:, :], rhs=hsum[:, :])
                ht = psm.tile([1, 1], F32, tag="ht", name=f"ht{b}")
                nc.vector.tensor_scalar(
                    ht[:, :],
                    tot_ps[:, :],
                    1.0 / (512 * (M - 1)),
                    1e-6,
                    op0=ALU.mult,
                    op1=ALU.max,
                )
                h2 = psm.tile([1, 1], F32, tag="h2", name=f"h2{b}")
                nc.scalar.activation(h2[:, :], ht[:, :], AF.Square)
                rh2 = psm.tile([1, 1], F32, tag="rh2", name=f"rh2{b}")
                nc.vector.reciprocal(rh2[:, :], h2[:, :])
                f_ps = pp.tile([128, 1], F32, tag="big", name=f"fps{b}")
                nc.tensor.matmul(f_ps[:, :], lhsT=ones_row[:, :], rhs=rh2[:, :])
                f_bc = psm.tile([128, 1], F32, tag="fbc", name=f"fbc{b}")
                nc.scalar.copy(f_bc[:, :], f_ps[:, :])
                # exp arg = f2*d2' - f2*sq_i/2  (the dropped EPS is a global
                # factor on every rho -> cannot change any comparison)
                bias_e = psm.tile([128, NT], F32, tag="bias_e", name=f"biase{b}")
                nc.vector.tensor_scalar(
                    bias_e[:, :], s["sqc"][:, :], f_bc[:, 0:1], -0.5,
                    op0=ALU.mult, op1=ALU.mult,
                )
                s["f_bc"], s["bias_e"] = f_bc, bias_e

            # ---------- phase: pass C (rho row-sums; exp/ln table set) --------
            for b in range(NB):
                s = st[b]
                rho = psm.tile([128, NT], F32, tag="rho", name=f"rho{b}")
                for t in range(NT):
                    d2_ps = pp.tile([128, M], F32, tag="big", name=f"d2c{b}_{t}")
                    emit_d2(s, t, d2_ps)
                    scr = pb.tile([128, M], BF16, tag="scr", bufs=1, name=f"sc{b}_{t}")
                    nc.scalar.activation(
                        scr[:, :],
                        d2_ps[:, :],
                        AF.Exp,
                        scale=s["f_bc"][:, :],
                        bias=s["bias_e"][:, t : t + 1],
                        accum_out=rho[:, t : t + 1],
                    )
                s["rho"] = rho

            # ---------- per batch building blocks ----------
            def emit_layout(b):
                """rho as an exact [1, M] row and [128, M] broadcast tile."""
                s = st[b]
                rT_ps = pp.tile([NT, 128], F32, tag="big", name=f"rTps{b}")
                nc.tensor.transpose(rT_ps[:, :], s["rho"][:, :], ident[:, :])
                rT = psm.tile([NT, 128], F32, tag="rT", name=f"rT{b}")
                nc.scalar.copy(rT[:, :], rT_ps[:, :])
                rrow = psm.tile([1, M], F32, tag="rowtmp", bufs=2, name=f"rrow{b}")
                nc.sync.dma_start(rrow[0:1, :], rT[:, :])
                rbc_ps = pp.tile([128, M], F32, tag="big", name=f"rbcps{b}")
                for j in range(NS):
                    nc.tensor.matmul(
                        rbc_ps[:, _sl(j)], lhsT=ones_row[:, :], rhs=rrow[0:1, _sl(j)]
                    )
                rho_bc = pb.tile([128, M], F32, tag="rho_bc", bufs=2, name=f"rbc{b}")
                nc.scalar.copy(rho_bc[:, :], rbc_ps[:, :])
                s["rho_bc"] = rho_bc

            def emit_gstream(b):
                """ranks r_i = #{k: rho_k < rho_i} via compare tiles (DVE +
                GpSimd) column-summed on the PE, then moved to chunk layout."""
                s = st[b]
                r_ps = pp.tile([1, M], F32, tag="big", name=f"rps{b}")
                for t in range(NT):
                    lt = pb.tile([128, M], F32R, tag="lt", bufs=3,
                                 name=f"lg_{b}_{t}")
                    eng = nc.gpsimd if t % 2 == 0 else nc.vector
                    eng.tensor_scalar(
                        lt[:, :], s["rho_bc"][:, :], s["rho"][:, t : t + 1], None,
                        op0=ALU.is_gt,
                    )
                    for j in range(NS):
                        nc.tensor.matmul(
                            r_ps[:, _sl(j)], lhsT=ones_col_r[:, 0:1],
                            rhs=lt[:, _sl(j)],
                            start=(t == 0), stop=(t == NT - 1),
                        )
                r_sb = psm.tile([1, M], F32, tag="rowtmp", bufs=2, name=f"rsb{b}")
                nc.scalar.copy(r_sb[0:1, :], r_ps[:, :])
                r16 = psm.tile([NT, 128], F32, tag="r16", name=f"r16_{b}")
                nc.sync.dma_start(r16[:, :], r_sb[0:1, :])
                rc_ps = pp.tile([128, NT], F32, tag="big", name=f"rcps{b}")
                nc.tensor.transpose(rc_ps[:, :], r16[:, :], ident[0:NT, 0:NT])
                racc = psm.tile([128, NT], F32, tag="racc", name=f"racc{b}")
                nc.scalar.copy(racc[:, :], rc_ps[:, :])
                s["racc"] = racc

            def emit_scalars_uv(b):
                """Dv/dvs/C vectors from ranks; u, v, T; dvs broadcasts."""
                s = st[b]
                racc = s["racc"]
                Dv = psm.tile([128, NT], F32, tag="Dv", name=f"Dv{b}")
                nc.vector.tensor_scalar_add(Dv[:, :], racc[:, :], 1.0)
                lnDv = psm.tile([128, NT], F32, tag="lnDv", name=f"lnDv{b}")
                nc.scalar.activation(lnDv[:, :], Dv[:, :], AF.Ln)
                dvs = psm.tile([128, NT], F32, tag="dvs", name=f"dvs{b}")
                nc.scalar.activation(dvs[:, :], lnDv[:, :], AF.Exp, scale=-0.5)
                g = psm.tile([128, NT], F32, tag="g", name=f"g{b}")
                nc.vector.tensor_scalar(
                    g[:, :], racc[:, :], -1.0, float(M - 1), op0=ALU.mult,
                    op1=ALU.add,
                )
                gm = psm.tile([128, NT], F32, tag="gm", name=f"gm{b}")
                nc.vector.tensor_scalar_max(gm[:, :], g[:, :], 1.0)
                inv = psm.tile([128, NT], F32, tag="inv", name=f"inv{b}")
                nc.vector.reciprocal(inv[:, :], gm[:, :])
                lng = psm.tile([128, NT], F32, tag="lng", name=f"lng{b}")
                nc.scalar.activation(lng[:, :], gm[:, :], AF.Ln)
                inv2 = psm.tile([128, NT], F32, tag="inv2", name=f"inv2{b}")
                nc.scalar.activation(inv2[:, :], inv[:, :], AF.Square)
                c1 = psm.tile([128, NT], F32, tag="c1", name=f"c1{b}")
                nc.vector.tensor_scalar(
                    c1[:, :], lng[:, :], -1.0, H_M - GAMMA, op0=ALU.mult, op1=ALU.add
                )
                c2 = psm.tile([128, NT], F32, tag="c2", name=f"c2{b}")
                nc.vector.scalar_tensor_tensor(
                    c2[:, :], in0=inv[:, :], scalar=-0.5, in1=c1[:, :],
                    op0=ALU.mult, op1=ALU.add,
                )
                Cv = psm.tile([128, NT], F32, tag="Cv", name=f"Cv{b}")
                nc.vector.scalar_tensor_tensor(
                    Cv[:, :], in0=inv2[:, :], scalar=1.0 / 12.0, in1=c2[:, :],
                    op0=ALU.mult, op1=ALU.add,
                )
                dvsC = psm.tile([128, NT], F32, tag="dvsC", name=f"dvsC{b}")
                nc.vector.tensor_mul(dvsC[:, :], dvs[:, :], Cv[:, :])

                u = pb.tile([128, M], F32R, tag="u", bufs=1, name=f"u{b}")
                v = pb.tile([128, M], F32R, tag="v", bufs=1, name=f"v{b}")
                for t in range(NT):
                    nc.vector.tensor_scalar(
                        u[:, _tb(t)], s["xn"][:, _tb(t)], dvs[:, t : t + 1], None,
                        op0=ALU.mult,
                    )
                    nc.vector.tensor_scalar(
                        v[:, _tb(t)], s["xn"][:, _tb(t)], dvsC[:, t : t + 1], None,
                        op0=ALU.mult,
                    )
                s["u"], s["v"] = u, v
                T_ps = pp.tile([128, 2], F32, tag="big", name=f"Tps{b}")
                for t in range(NT):
                    nc.tensor.matmul(
                        T_ps[:, :], lhsT=u[:, _tb(t)], rhs=ones_col_r[:, :],
                        start=(t == 0), stop=(t == NT - 1),
                    )
                T_sb = psm.tile([128, 1], F32, tag="T_sb", name=f"Tsb{b}")
                nc.scalar.copy(T_sb[:, :], T_ps[:, 0:1])
                s["T_sb"] = T_sb

                stk = psm.tile([128, 2 * NT], F32, tag="stk", name=f"stk{b}")
                nc.vector.tensor_copy(stk[:, 0:NT], dvs[:, :])
                nc.vector.tensor_copy(stk[:, NT : 2 * NT], dvsC[:, :])
                stT_ps = pp.tile([2 * NT, 128], F32, tag="big", name=f"stTps{b}")
                nc.tensor.transpose(stT_ps[:, :], stk[:, :], ident[:, :])
                stT = psm.tile([2 * NT, 128], F32R, tag="stT", name=f"stT{b}")
                nc.vector.tensor_copy(stT[:, :], stT_ps[:, :])
                dvs_row = psm.tile([1, M], F32R, tag="rowtmp", bufs=2,
                                   name=f"dr{b}")
                nc.sync.dma_start(dvs_row[0:1, :], stT[0:NT, :])
                dvsC_row = psm.tile([1, M], F32R, tag="rowtmp", bufs=2,
                                    name=f"cr{b}")
                nc.sync.dma_start(dvsC_row[0:1, :], stT[NT : 2 * NT, :])

                dbc_ps = pp.tile([128, M], F32, tag="big", name=f"dbcps{b}")
                for j in range(NS):
                    nc.tensor.matmul(
                        dbc_ps[:, _sl(j)], lhsT=ones_row_r[:, :],
                        rhs=dvs_row[0:1, _sl(j)],
                    )
                dvs_bc = pb.tile([128, M], F32, tag="dvs_bc", bufs=1, name=f"db{b}")
                nc.scalar.copy(dvs_bc[:, :], dbc_ps[:, :])
                cbc_ps = pp.tile([128, M], F32, tag="big", name=f"cbcps{b}")
                for j in range(NS):
                    nc.tensor.matmul(
                        cbc_ps[:, _sl(j)], lhsT=ones_row_r[:, :],
                        rhs=dvsC_row[0:1, _sl(j)],
                    )
                dvsC_bc = pb.tile([128, M], F32, tag="dvsC_bc", bufs=1,
                                  name=f"cb{b}")
                nc.scalar.copy(dvsC_bc[:, :], cbc_ps[:, :])
                s["dvs_bc"], s["dvsC_bc"] = dvs_bc, dvsC_bc

            def emit_l2(b):
                """L pass 2 + P1 = (L@v)^T, P2 = (L@u)^T ([c, i] PSUM)."""
                s = st[b]
                P1_ps = pp.tile([128, M], F32, tag="big", name=f"P1ps{b}")
                P2_ps = pp.tile([128, M], F32, tag="big", name=f"P2ps{b}")
                for t in range(NT):
                    lt = pb.tile([128, M], F32R, tag="lt", bufs=3,
                                 name=f"lt2_{b}_{t}")
                    eng = nc.gpsimd if t % 3 == 0 else nc.vector
                    eng.tensor_scalar(
                        lt[:, :], s["rho_bc"][:, :], s["rho"][:, t : t + 1], None,
                        op0=ALU.is_gt,
                    )
                    for j in range(NS):
                        nc.tensor.matmul(
                            P2_ps[:, _sl(j)], lhsT=s["u"][:, _tb(t)],
                            rhs=lt[:, _sl(j)],
                            start=(t == 0), stop=(t == NT - 1),
                        )
                    for j in range(NS):
                        nc.tensor.matmul(
                            P1_ps[:, _sl(j)], lhsT=s["v"][:, _tb(t)],
                            rhs=lt[:, _sl(j)],
                            start=(t == 0), stop=(t == NT - 1),
                        )
                s["P1_ps"], s["P2_ps"] = P1_ps, P2_ps

            def emit_z(b):
                """z = dvs_i*P1 - (dvs*C)_i*(P2 - T_c)   ([c, i] layout)."""
                s = st[b]
                zt1 = pb.tile([128, M], F32, tag="v", bufs=1, name=f"zt1{b}")
                nc.vector.scalar_tensor_tensor(
                    zt1[:, :], in0=s["P2_ps"][:, :], scalar=s["T_sb"][:, 0:1],
                    in1=s["dvsC_bc"][:, :], op0=ALU.subtract, op1=ALU.mult,
                )
                zt2 = pb.tile([128, M], F32, tag="u", bufs=1, name=f"zt2{b}")
                nc.vector.scalar_tensor_tensor(
                    zt2[:, :], in0=s["P1_ps"][:, :], scalar=0.0,
                    in1=s["dvs_bc"][:, :], op0=ALU.bypass, op1=ALU.mult,
                )
                z = pb.tile([128, M], F32R, tag="z", name=f"z{b}")
                nc.vector.tensor_sub(z[:, :], zt2[:, :], zt1[:, :])
                s["z"] = z

            def emit_proj(b):
                """yT = (W @ z) with constant stationary; SiLU; store [o, i]."""
                s = st[b]
                yT_ps = pp.tile([128, M], F32, tag="big", name=f"yTps{b}")
                for j in range(NS):
                    nc.tensor.matmul(
                        yT_ps[:, _sl(j)], lhsT=wt_r[:, :], rhs=s["z"][:, _sl(j)]
                    )
                sg = pb.tile([128, M], F32, tag="sg", bufs=1, name=f"sgp{b}")
                nc.scalar.activation(sg[:, :], yT_ps[:, :], AF.Sigmoid)
                y_sb = pb.tile([128, M], F32, tag="y_sb", bufs=1, name=f"ysb{b}")
                nc.vector.tensor_mul(y_sb[:, :], yT_ps[:, :], sg[:, :])
                nc.sync.dma_start(yH[b], y_sb[:, :])

            # ---------- schedule: interleave batches for engine overlap ------
            emit_layout(0)
            emit_layout(1)
            emit_gstream(0)
            emit_scalars_uv(0)
            emit_l2(0)
            emit_z(0)
            emit_gstream(1)
            emit_scalars_uv(1)
            emit_l2(1)
            emit_z(1)
            emit_proj(0)
            emit_proj(1)

    nc.compile()
    return nc


_CACHED_NC = None


def _get_nc():
    global _CACHED_NC
    if _CACHED_NC is None:
        _CACHED_NC = build_kernel()
    return _CACHED_NC


def make_in_maps(x, W):
    x = np.asarray(x, dtype=np.float32)
    W = np.asarray(W, dtype=np.float32)
    wt = np.ascontiguousarray(W.T)
    in_maps = []
    for core in range(N_CORES):
        xb = x[core * NB : (core + 1) * NB]                       # [NB, M, C]
        xt = np.ascontiguousarray(xb.transpose(0, 2, 1))          # [NB, C, M]
        # xn[b, p, t*128+c] = x[b, t*128+p, c]
        xn = np.ascontiguousarray(
            xb.reshape(NB, NT, 128, C).transpose(0, 2, 1, 3).reshape(NB, 128, M)
        )
        in_maps.append({"xT": xt, "xN": xn, "WT": wt})
    return in_maps


def unshard_output(results):
    outs = []
    for core in range(N_CORES):
        yh = results[core]["yH"]                                  # [NB, C, M]
        outs.append(yh.transpose(0, 2, 1))                        # [NB, M, C]
    return np.concatenate(outs, axis=0).astype(np.float32)


def run(x, W, trace=False, trace_kwargs=None):
    nc = _get_nc()
    res = run_bass_kernel_spmd(
        nc,
        make_in_maps(x, W),
        list(range(N_CORES)),
        trace=trace,
        **(trace_kwargs or {}),
    )
    return unshard_output(res.results), res


def kernel(x, W):
    y, _ = run(x, W, trace=False)
    return y


# revision 20
# speedup vs baseline: 3.0751x; 3.0751x over previous
"""KDE HyperGraph Conv kernel for 8 Trainium2 NeuronCores.

Math: the reference builds H[i,j] = [rho_i > rho_j] (+ self loop), so every
downstream quantity depends only on the *ranks* of the KDE densities rho.
With r_i = rank(rho_i) (ascending) and g_i = M-1-r_i (# of larger rhos):

    De_j = g_j + 1,  Dv_i = M - g_i
    A[i,k] = C(min(r_i, r_k)),  C(r) = H_M - H_{M-1-r}   (harmonic numbers)
    y_i = dvs_i * (  sum_{r_k < r_i} C_k dvs_k x_k
                   + C_i * sum_{r_k >= r_i} dvs_k x_k ),  dvs = Dv^-1/2

which turns the [M,M]@[M,M] propagation into two [M,M]@[M,C] matmuls with
the 0/1 comparison matrix L[i,k] = [rho_k < rho_i] plus elementwise work.
C(r) is evaluated with the asymptotic expansion of harmonic numbers.

Distance pass: PE computes d2'[i,j] = x_i.x_j - sq_j/2 into PSUM; the
ScalarEngine applies scale/bias (adding sq_i per-partition) while doing
sqrt / exp, with a free row-sum (accum_out) giving h and rho directly.

Data-parallel over batch: each of the 8 cores handles B/8 = 2 batches.
"""

import os
import sys

for _p in ("/opt/trn_rl_repo",):
    if os.path.isdir(_p) and _p not in sys.path:
        sys.path.append(_p)

import numpy as np

import concourse.bass as bass
import concourse.tile as tile
from concourse import bacc, mybir
from concourse.bass_utils import run_bass_kernel_spmd
from concourse.masks import make_identity

N_CORES = 8
B, M, C = 16, 2048, 128
NB = B // N_CORES          # batches per core
NT = M // 128              # 128-row chunks per batch
NS = M // 512              # 512-wide column slices
EPS = 0.1                  # diagonal-safety shift added inside sqrt
                           # (covers float32r rounding noise on d2 diag)
GAMMA = 0.5772156649015329
H_M = float((1.0 / np.arange(1, M + 1, dtype=np.float64)).sum())

F32 = mybir.dt.float32
BF16 = mybir.dt.bfloat16
F32R = mybir.dt.float32r
AF = mybir.ActivationFunctionType
ALU = mybir.AluOpType
AX = mybir.AxisListType


def _tb(t):
    return slice(t * 128, (t + 1) * 128)


def _sl(j):
    return slice(j * 512, (j + 1) * 512)


def build_kernel():
    nc = bacc.Bacc("TRN2", target_bir_lowering=False, debug=False)

    # Per-core inputs (host pre-permuted, see make_in_maps below):
    #   xT[b, c, i]          = x[b, i, c]            (channels on partitions)
    #   xN[b, p, t*128 + c]  = x[b, t*128 + p, c]    (rows on partitions, chunked)
    #   WT[c, o]             = W[o, c]               (projection, pre-transposed)
    # Output yH[b, o, i] = y[b, i, o]  (transposed; host un-permutes)
    xT = nc.declare_dram_parameter("xT", [NB, 128, M], F32, isOutput=False)
    xN = nc.declare_dram_parameter("xN", [NB, 128, M], F32, isOutput=False)
    WT = nc.declare_dram_parameter("WT", [128, 128], F32, isOutput=False)
    yH = nc.declare_dram_parameter("yH", [NB, 128, M], F32, isOutput=True)

    with tile.TileContext(nc) as tc:
        from contextlib import ExitStack

        with ExitStack() as ctx:
            cp = ctx.enter_context(tc.tile_pool(name="consts", bufs=1))
            pb = ctx.enter_context(tc.tile_pool(name="big", bufs=2))
            psm = ctx.enter_context(tc.tile_pool(name="small", bufs=2))
            pp = ctx.enter_context(
                tc.tile_pool(name="psum", bufs=2, space=bass.MemorySpace.PSUM)
            )

            ones_col = cp.tile([128, 1], F32, tag="ones_col")
            nc.gpsimd.memset(ones_col[:, :], 1.0)
            ones_col_r = cp.tile([128, 2], F32R, tag="ones_col_r")
            nc.vector.tensor_copy(ones_col_r[:, 0:1], ones_col[:, :])
            nc.vector.tensor_copy(ones_col_r[:, 1:2], ones_col[:, :])
            ones_row = cp.tile([1, 128], F32, tag="ones_row")
            nc.gpsimd.memset(ones_row[:, :], 1.0)
            ones_row_r = cp.tile([1, 128], F32R, tag="ones_row_r")
            nc.vector.tensor_copy(ones_row_r[:, :], ones_row[:, :])
            mhalf_f32 = cp.tile([1, 128], F32, tag="mhalf_f32")
            nc.gpsimd.memset(mhalf_f32[:, :], -0.5)
            mhalf_row = cp.tile([1, 128], F32R, tag="mhalf_row")
            nc.vector.tensor_copy(mhalf_row[:, :], mhalf_f32[:, :])
            ident = cp.tile([128, 128], F32, tag="ident")
            make_identity(nc, ident[:, :])
            wt_sb = cp.tile([128, 128], F32, tag="wt")
            nc.sync.dma_start(wt_sb[:, :], WT[:, :])
            wt_r = cp.tile([128, 128], F32R, tag="wt_r")
            nc.vector.tensor_copy(wt_r[:, :], wt_sb[:, :])

            st = [dict() for _ in range(NB)]

            # ---------- phase: prep (loads, squares, sq row + chunk) ----------
            for b in range(NB):
                s = st[b]
                xt = pb.tile([128, M], F32, tag="xt", bufs=1, name=f"xt{b}")
                nc.sync.dma_start(xt[:, :], xT[b])
                xt_r = pb.tile([128, M], F32R, tag="xt_r", name=f"xtr{b}")
                nc.vector.tensor_copy(xt_r[:, :], xt[:, :])
                xn = pb.tile([128, M], F32, tag="xn", name=f"xn{b}")
                nc.sync.dma_start(xn[:, :], xN[b])
                xsq = pb.tile([128, M], F32R, tag="xsq", bufs=1, name=f"xsq{b}")
                nc.scalar.activation(xsq[:, :], xt_r[:, :], AF.Square)
                sq_ps = pp.tile([1, M], F32, tag="big", name=f"sqps{b}")
                for j in range(NS):
                    nc.tensor.matmul(
                        sq_ps[:, _sl(j)], lhsT=ones_col_r[:, 0:1], rhs=xsq[:, _sl(j)]
                    )
                sq_row = psm.tile([1, M], F32R, tag="sq_row", name=f"sqrow{b}")
                nc.scalar.copy(sq_row[0:1, :], sq_ps[:, :])
                # sq in chunk layout [128, NT] via DMA + PE transpose so the
                # ACT bias sees the same PE-computed sq as the PSUM d2'
                s16 = psm.tile([NT, 128], F32, tag="s16", name=f"s16_{b}")
                nc.sync.dma_start(s16[:, :], sq_row[0:1, :].bitcast(F32))
                sqc_ps = pp.tile([128, NT], F32, tag="big", name=f"sqcps{b}")
                nc.tensor.transpose(sqc_ps[:, :], s16[:, :], ident[0:NT, 0:NT])
                sqc = psm.tile([128, NT], F32, tag="sqc", name=f"sqc{b}")
                nc.scalar.copy(sqc[:, :], sqc_ps[:, :])
                bias_s = psm.tile([128, NT], F32, tag="bias_s", name=f"biass{b}")
                nc.vector.tensor_scalar_add(bias_s[:, :], sqc[:, :], EPS)
                s["xt_r"], s["xn"], s["sq_row"], s["sqc"], s["bias_s"] = (
                    xt_r, xn, sq_row, sqc, bias_s,
                )

            def emit_d2(s, t, d2_ps):
                # PSUM d2'[i, j] = x_i.x_j - sq_j/2   (i in chunk t)
                # full d2 = -2*d2' + sq_i (+EPS), applied via ACT scale/bias
                for j in range(NS):
                    nc.tensor.matmul(
                        d2_ps[:, _sl(j)],
                        lhsT=mhalf_row[0:1, :],
                        rhs=s["sq_row"][0:1, _sl(j)],
                        start=True,
                        stop=False,
                    )
                for j in range(NS):
                    nc.tensor.matmul(
                        d2_ps[:, _sl(j)],
                        lhsT=s["xt_r"][:, _tb(t)],
                        rhs=s["xt_r"][:, _sl(j)],
                        start=False,
                        stop=True,
                    )

            # ---------- phase: pass B (sampled dist sums for h; sqrt set) ----
            # h is a mean over 4.2M off-diag distances; a 4-tile (512-row)
            # sample estimates it to ~4e-5 relative, far below the spacing
            # of adjacent rho values, so ranks are unaffected.
            HS = (0, 4, 8, 12)
            for b in range(NB):
                s = st[b]
                hacc = psm.tile([128, len(HS)], F32, tag="hacc", name=f"hacc{b}")
                for hi, t in enumerate(HS):
                    d2_ps = pp.tile([128, M], F32, tag="big", name=f"d2b{b}_{t}")
                    emit_d2(s, t, d2_ps)
                    scr = pb.tile([128, M], BF16, tag="scr", bufs=1, name=f"sb{b}_{t}")
                    nc.scalar.activation(
                        scr[:, :],
                        d2_ps[:, :],
                        AF.Sqrt,
                        scale=-2.0,
                        bias=s["bias_s"][:, t : t + 1],
                        accum_out=hacc[:, hi : hi + 1],
                    )
                s["hacc"] = hacc

            # ---------- phase: h -> f2 = 1/h^2, exp bias -------------------
            for b in range(NB):
                s = st[b]
                hsum = psm.tile([128, 1], F32, tag="hsum", name=f"hsum{b}")
                nc.vector.tensor_reduce(
                    hsum[:, :], s["hacc"][:, :], axis=AX.X, op=ALU.add
                )
                tot_ps = pp.tile([1, 1], F32, tag="big", name=f"tot{b}")
                nc.tensor.matmul(tot_ps[:, :], lhsT=ones_col[:, :], rhs=hsum[:, :])
                ht = psm.tile([1, 1], F32, tag="ht", name=f"ht{b}")
                nc.vector.tensor_scalar(
                    ht[:, :],
                    tot_ps[:, :],
                    1.0 / (512 * (M - 1)),
                    1e-6,
                    op0=ALU.mult,
                    op1=ALU.max,
                )
                h2 = psm.tile([1, 1], F32, tag="h2", name=f"h2{b}")
                nc.scalar.activation(h2[:, :], ht[:, :], AF.Square)
                rh2 = psm.tile([1, 1], F32, tag="rh2", name=f"rh2{b}")
                nc.vector.reciprocal(rh2[:, :], h2[:, :])
                f_ps = pp.tile([128, 1], F32, tag="big", name=f"fps{b}")
                nc.tensor.matmul(f_ps[:, :], lhsT=ones_row[:, :], rhs=rh2[:, :])
                f_bc = psm.tile([128, 1], F32, tag="fbc", name=f"fbc{b}")
                nc.scalar.copy(f_bc[:, :], f_ps[:, :])
                # exp arg = f2*d2' - f2*sq_i/2  (the dropped EPS is a global
                # factor on every rho -> cannot change any comparison)
                bias_e = psm.tile([128, NT], F32, tag="bias_e", name=f"biase{b}")
                nc.vector.tensor_scalar(
                    bias_e[:, :], s["sqc"][:, :], f_bc[:, 0:1], -0.5,
                    op0=ALU.mult, op1=ALU.mult,
                )
                s["f_bc"], s["bias_e"] = f_bc, bias_e

            # ---------- phase: pass C (rho row-sums; exp/ln table set) --------
            for b in range(NB):
                s = st[b]
                rho = psm.tile([128, NT], F32, tag="rho", name=f"rho{b}")
                for t in range(NT):
                    d2_ps = pp.tile([128, M], F32, tag="big", name=f"d2c{b}_{t}")
                    emit_d2(s, t, d2_ps)
                    scr = pb.tile([128, M], BF16, tag="scr", bufs=1, name=f"sc{b}_{t}")
                    nc.scalar.activation(
                        scr[:, :],
                        d2_ps[:, :],
                        AF.Exp,
                        scale=s["f_bc"][:, :],
                        bias=s["bias_e"][:, t : t + 1],
                        accum_out=rho[:, t : t + 1],
                    )
                s["rho"] = rho

            # ---------- per batch building blocks ----------
            def emit_layout(b):
                """rho as an exact [1, M] row and [128, M] broadcast tile."""
                s = st[b]
                rT_ps = pp.tile([NT, 128], F32, tag="big", name=f"rTps{b}")
                nc.tensor.transpose(rT_ps[:, :], s["rho"][:, :], ident[:, :])
                rT = psm.tile([NT, 128], F32, tag="rT", name=f"rT{b}")
                nc.scalar.copy(rT[:, :], rT_ps[:, :])
                rrow = psm.tile([1, M], F32, tag="rowtmp", bufs=2, name=f"rrow{b}")
                nc.sync.dma_start(rrow[0:1, :], rT[:, :])
                rbc_ps = pp.tile([128, M], F32, tag="big", name=f"rbcps{b}")
                for j in range(NS):
                    nc.tensor.matmul(
                        rbc_ps[:, _sl(j)], lhsT=ones_row[:, :], rhs=rrow[0:1, _sl(j)]
                    )
                rho_bc = pb.tile([128, M], F32, tag="rho_bc", bufs=2, name=f"rbc{b}")
                nc.scalar.copy(rho_bc[:, :], rbc_ps[:, :])
                s["rho_bc"] = rho_bc

            def emit_gstream(b):
                """ranks r_i = #{k: rho_k < rho_i} via compare tiles (DVE +
                GpSimd) column-summed on the PE, then moved to chunk layout."""
                s = st[b]
                r_ps = pp.tile([1, M], F32, tag="big", name=f"rps{b}")
                for t in range(NT):
                    lt = pb.tile([128, M], F32R, tag="lt", bufs=3,
                                 name=f"lg_{b}_{t}")
                    nc.vector.tensor_scalar(
                        lt[:, :], s["rho_bc"][:, :], s["rho"][:, t : t + 1], None,
                        op0=ALU.is_gt,
                    )
                    for j in range(NS):
                        nc.tensor.matmul(
                            r_ps[:, _sl(j)], lhsT=ones_col_r[:, 0:1],
                            rhs=lt[:, _sl(j)],
                            start=(t == 0), stop=(t == NT - 1),
                        )
                r_sb = psm.tile([1, M], F32, tag="rowtmp", bufs=2, name=f"rsb{b}")
                nc.scalar.copy(r_sb[0:1, :], r_ps[:, :])
                r16 = psm.tile([NT, 128], F32, tag="r16", name=f"r16_{b}")
                nc.sync.dma_start(r16[:, :], r_sb[0:1, :])
                rc_ps = pp.tile([128, NT], F32, tag="big", name=f"rcps{b}")
                nc.tensor.transpose(rc_ps[:, :], r16[:, :], ident[0:NT, 0:NT])
                racc = psm.tile([128, NT], F32, tag="racc", name=f"racc{b}")
                nc.scalar.copy(racc[:, :], rc_ps[:, :])
                s["racc"] = racc

            def emit_scalars_uv(b):
                """Dv/dvs/C vectors from ranks; u, v, T; dvs broadcasts."""
                s = st[b]
                racc = s["racc"]
                Dv = psm.tile([128, NT], F32, tag="Dv", name=f"Dv{b}")
                nc.vector.tensor_scalar_add(Dv[:, :], racc[:, :], 1.0)
                lnDv = psm.tile([128, NT], F32, tag="lnDv", name=f"lnDv{b}")
                nc.scalar.activation(lnDv[:, :], Dv[:, :], AF.Ln)
                dvs = psm.tile([128, NT], F32, tag="dvs", name=f"dvs{b}")
                nc.scalar.activation(dvs[:, :], lnDv[:, :], AF.Exp, scale=-0.5)
                g = psm.tile([128, NT], F32, tag="g", name=f"g{b}")
                nc.vector.tensor_scalar(
                    g[:, :], racc[:, :], -1.0, float(M - 1), op0=ALU.mult,
                    op1=ALU.add,
                )
                gm = psm.tile([128, NT], F32, tag="gm", name=f"gm{b}")
                nc.vector.tensor_scalar_max(gm[:, :], g[:, :], 1.0)
                inv = psm.tile([128, NT], F32, tag="inv", name=f"inv{b}")
                nc.vector.reciprocal(inv[:, :], gm[:, :])
                lng = psm.tile([128, NT], F32, tag="lng", name=f"lng{b}")
                nc.scalar.activation(lng[:, :], gm[:, :], AF.Ln)
                inv2 = psm.tile([128, NT], F32, tag="inv2", name=f"inv2{b}")
                nc.scalar.activation(inv2[:, :], inv[:, :], AF.Square)
                c1 = psm.tile([128, NT], F32, tag="c1", name=f"c1{b}")
                nc.vector.tensor_scalar(
                    c1[:, :], lng[:, :], -1.0, H_M - GAMMA, op0=ALU.mult, op1=ALU.add
                )
                c2 = psm.tile([128, NT], F32, tag="c2", name=f"c2{b}")
                nc.vector.scalar_tensor_tensor(
                    c2[:, :], in0=inv[:, :], scalar=-0.5, in1=c1[:, :],
                    op0=ALU.mult, op1=ALU.add,
                )
                Cv = psm.tile([128, NT], F32, tag="Cv", name=f"Cv{b}")
                nc.vector.scalar_tensor_tensor(
                    Cv[:, :], in0=inv2[:, :], scalar=1.0 / 12.0, in1=c2[:, :],
                    op0=ALU.mult, op1=ALU.add,
                )
                dvsC = psm.tile([128, NT], F32, tag="dvsC", name=f"dvsC{b}")
                nc.vector.tensor_mul(dvsC[:, :], dvs[:, :], Cv[:, :])

                u = pb.tile([128, M], F32R, tag="u", bufs=1, name=f"u{b}")
                v = pb.tile([128, M], F32R, tag="v", bufs=1, name=f"v{b}")
                for t in range(NT):
                    nc.vector.tensor_scalar(
                        u[:, _tb(t)], s["xn"][:, _tb(t)], dvs[:, t : t + 1], None,
                        op0=ALU.mult,
                    )
                    nc.vector.tensor_scalar(
                        v[:, _tb(t)], s["xn"][:, _tb(t)], dvsC[:, t : t + 1], None,
                        op0=ALU.mult,
                    )
                s["u"], s["v"] = u, v
                T_ps = pp.tile([128, 2], F32, tag="big", name=f"Tps{b}")
                for t in range(NT):
                    nc.tensor.matmul(
                        T_ps[:, :], lhsT=u[:, _tb(t)], rhs=ones_col_r[:, :],
                        start=(t == 0), stop=(t == NT - 1),
                    )
                T_sb = psm.tile([128, 1], F32, tag="T_sb", name=f"Tsb{b}")
                nc.scalar.copy(T_sb[:, :], T_ps[:, 0:1])
                s["T_sb"] = T_sb

                stk = psm.tile([128, 2 * NT], F32, tag="stk", name=f"stk{b}")
                nc.vector.tensor_copy(stk[:, 0:NT], dvs[:, :])
                nc.vector.tensor_copy(stk[:, NT : 2 * NT], dvsC[:, :])
                stT_ps = pp.tile([2 * NT, 128], F32, tag="big", name=f"stTps{b}")
                nc.tensor.transpose(stT_ps[:, :], stk[:, :], ident[:, :])
                stT = psm.tile([2 * NT, 128], F32R, tag="stT", name=f"stT{b}")
                nc.vector.tensor_copy(stT[:, :], stT_ps[:, :])
                dvs_row = psm.tile([1, M], F32R, tag="rowtmp", bufs=2,
                                   name=f"dr{b}")
                nc.sync.dma_start(dvs_row[0:1, :], stT[0:NT, :])
                dvsC_row = psm.tile([1, M], F32R, tag="rowtmp", bufs=2,
                                    name=f"cr{b}")
                nc.sync.dma_start(dvsC_row[0:1, :], stT[NT : 2 * NT, :])

                dbc_ps = pp.tile([128, M], F32, tag="big", name=f"dbcps{b}")
                for j in range(NS):
                    nc.tensor.matmul(
                        dbc_ps[:, _sl(j)], lhsT=ones_row_r[:, :],
                        rhs=dvs_row[0:1, _sl(j)],
                    )
                dvs_bc = pb.tile([128, M], F32, tag="dvs_bc", bufs=1, name=f"db{b}")
                nc.scalar.copy(dvs_bc[:, :], dbc_ps[:, :])
                cbc_ps = pp.tile([128, M], F32, tag="big", name=f"cbcps{b}")
                for j in range(NS):
                    nc.tensor.matmul(
                        cbc_ps[:, _sl(j)], lhsT=ones_row_r[:, :],
                        rhs=dvsC_row[0:1, _sl(j)],
                    )
                dvsC_bc = pb.tile([128, M], F32, tag="dvsC_bc", bufs=1,
                                  name=f"cb{b}")
                nc.scalar.copy(dvsC_bc[:, :], cbc_ps[:, :])
                s["dvs_bc"], s["dvsC_bc"] = dvs_bc, dvsC_bc

            def emit_l2(b):
                """L pass 2 + P1 = (L@v)^T, P2 = (L@u)^T ([c, i] PSUM)."""
                s = st[b]
                P1_ps = pp.tile([128, M], F32, tag="big", name=f"P1ps{b}")
                P2_ps = pp.tile([128, M], F32, tag="big", name=f"P2ps{b}")
                for t in range(NT):
                    lt = pb.tile([128, M], F32R, tag="lt", bufs=3,
                                 name=f"lt2_{b}_{t}")
                    nc.vector.tensor_scalar(
                        lt[:, :], s["rho_bc"][:, :], s["rho"][:, t : t + 1], None,
                        op0=ALU.is_gt,
                    )
                    for j in range(NS):
                        nc.tensor.matmul(
                            P2_ps[:, _sl(j)], lhsT=s["u"][:, _tb(t)],
                            rhs=lt[:, _sl(j)],
                            start=(t == 0), stop=(t == NT - 1),
                        )
                    for j in range(NS):
                        nc.tensor.matmul(
                            P1_ps[:, _sl(j)], lhsT=s["v"][:, _tb(t)],
                            rhs=lt[:, _sl(j)],
                            start=(t == 0), stop=(t == NT - 1),
                        )
                s["P1_ps"], s["P2_ps"] = P1_ps, P2_ps

            def emit_z(b):
                """z = dvs_i*P1 - (dvs*C)_i*(P2 - T_c)   ([c, i] layout)."""
                s = st[b]
                zt1 = pb.tile([128, M], F32, tag="v", bufs=1, name=f"zt1{b}")
                nc.vector.scalar_tensor_tensor(
                    zt1[:, :], in0=s["P2_ps"][:, :], scalar=s["T_sb"][:, 0:1],
                    in1=s["dvsC_bc"][:, :], op0=ALU.subtract, op1=ALU.mult,
                )
                zt2 = pb.tile([128, M], F32, tag="u", bufs=1, name=f"zt2{b}")
                nc.vector.scalar_tensor_tensor(
                    zt2[:, :], in0=s["P1_ps"][:, :], scalar=0.0,
                    in1=s["dvs_bc"][:, :], op0=ALU.bypass, op1=ALU.mult,
                )
                z = pb.tile([128, M], F32R, tag="z", name=f"z{b}")
                nc.vector.tensor_sub(z[:, :], zt2[:, :], zt1[:, :])
                s["z"] = z

            def emit_proj(b):
                """yT = (W @ z) with constant stationary; SiLU; store [o, i]."""
                s = st[b]
                yT_ps = pp.tile([128, M], F32, tag="big", name=f"yTps{b}")
                for j in range(NS):
                    nc.tensor.matmul(
                        yT_ps[:, _sl(j)], lhsT=wt_r[:, :], rhs=s["z"][:, _sl(j)]
                    )
                sg = pb.tile([128, M], F32, tag="sg", bufs=1, name=f"sgp{b}")
                nc.scalar.activation(sg[:, :], yT_ps[:, :], AF.Sigmoid)
                y_sb = pb.tile([128, M], F32, tag="y_sb", bufs=1, name=f"ysb{b}")
                nc.vector.tensor_mul(y_sb[:, :], yT_ps[:, :], sg[:, :])
                nc.sync.dma_start(yH[b], y_sb[:, :])

            # ---------- schedule: interleave batches for engine overlap ------
            emit_layout(0)
            emit_layout(1)
            emit_gstream(0)
            emit_scalars_uv(0)
            emit_l2(0)
            emit_z(0)
            emit_gstream(1)
            emit_scalars_uv(1)
            emit_l2(1)
            emit_z(1)
            emit_proj(0)
            emit_proj(1)

    nc.compile()
    return nc


_CACHED_NC = None


def _get_nc():
    global _CACHED_NC
    if _CACHED_NC is None:
        _CACHED_NC = build_kernel()
    return _CACHED_NC


def make_in_maps(x, W):
    x = np.asarray(x, dtype=np.float32)
    W = np.asarray(W, dtype=np.float32)
    wt = np.ascontiguousarray(W.T)
    in_maps = []
    for core in range(N_CORES):
        xb = x[core * NB : (core + 1) * NB]                       # [NB, M, C]
        xt = np.ascontiguousarray(xb.transpose(0, 2, 1))          # [NB, C, M]
        # xn[b, p, t*128+c] = x[b, t*128+p, c]
        xn = np.ascontiguousarray(
            xb.reshape(NB, NT, 128, C).transpose(0, 2, 1, 3).reshape(NB, 128, M)
        )
        in_maps.append({"xT": xt, "xN": xn, "WT": wt})
    return in_maps


def unshard_output(results):
    outs = []
    for core in range(N_CORES):
        yh = results[core]["yH"]                                  # [NB, C, M]
        outs.append(yh.transpose(0, 2, 1))                        # [NB, M, C]
    return np.concatenate(outs, axis=0).astype(np.float32)


def run(x, W, trace=False, trace_kwargs=None):
    nc = _get_nc()
    res = run_bass_kernel_spmd(
        nc,
        make_in_maps(x, W),
        list(range(N_CORES)),
        trace=trace,
        **(trace_kwargs or {}),
    )
    return unshard_output(res.results), res


def kernel(x, W):
    y, _ = run(x, W, trace=False)
    return y


# revision 21
# speedup vs baseline: 3.0868x; 1.0038x over previous
"""KDE HyperGraph Conv kernel for 8 Trainium2 NeuronCores.

Math: the reference builds H[i,j] = [rho_i > rho_j] (+ self loop), so every
downstream quantity depends only on the *ranks* of the KDE densities rho.
With r_i = rank(rho_i) (ascending) and g_i = M-1-r_i (# of larger rhos):

    De_j = g_j + 1,  Dv_i = M - g_i
    A[i,k] = C(min(r_i, r_k)),  C(r) = H_M - H_{M-1-r}   (harmonic numbers)
    y_i = dvs_i * (  sum_{r_k < r_i} C_k dvs_k x_k
                   + C_i * sum_{r_k >= r_i} dvs_k x_k ),  dvs = Dv^-1/2

which turns the [M,M]@[M,M] propagation into two [M,M]@[M,C] matmuls with
the 0/1 comparison matrix L[i,k] = [rho_k < rho_i] plus elementwise work.
C(r) is evaluated with the asymptotic expansion of harmonic numbers.

Distance pass: PE computes d2'[i,j] = x_i.x_j - sq_j/2 into PSUM; the
ScalarEngine applies scale/bias (adding sq_i per-partition) while doing
sqrt / exp, with a free row-sum (accum_out) giving h and rho directly.

Data-parallel over batch: each of the 8 cores handles B/8 = 2 batches.
"""

import os
import sys

for _p in ("/opt/trn_rl_repo",):
    if os.path.isdir(_p) and _p not in sys.path:
        sys.path.append(_p)

import numpy as np

import concourse.bass as bass
import concourse.tile as tile
from concourse import bacc, mybir
from concourse.bass_utils import run_bass_kernel_spmd
from concourse.masks import make_identity

N_CORES = 8
B, M, C = 16, 2048, 128
NB = B // N_CORES          # batches per core
NT = M // 128              # 128-row chunks per batch
NS = M // 512              # 512-wide column slices
EPS = 0.1                  # diagonal-safety shift added inside sqrt
                           # (covers float32r rounding noise on d2 diag)
GAMMA = 0.5772156649015329
H_M = float((1.0 / np.arange(1, M + 1, dtype=np.float64)).sum())

F32 = mybir.dt.float32
BF16 = mybir.dt.bfloat16
F32R = mybir.dt.float32r
AF = mybir.ActivationFunctionType
ALU = mybir.AluOpType
AX = mybir.AxisListType


def _tb(t):
    return slice(t * 128, (t + 1) * 128)


def _sl(j):
    return slice(j * 512, (j + 1) * 512)


def build_kernel():
    nc = bacc.Bacc("TRN2", target_bir_lowering=False, debug=False)

    # Per-core inputs (host pre-permuted, see make_in_maps below):
    #   xT[b, c, i]          = x[b, i, c]            (channels on partitions)
    #   xN[b, p, t*128 + c]  = x[b, t*128 + p, c]    (rows on partitions, chunked)
    #   WT[c, o]             = W[o, c]               (projection, pre-transposed)
    # Output yH[b, o, i] = y[b, i, o]  (transposed; host un-permutes)
    xT = nc.declare_dram_parameter("xT", [NB, 128, M], F32, isOutput=False)
    xN = nc.declare_dram_parameter("xN", [NB, 128, M], F32, isOutput=False)
    WT = nc.declare_dram_parameter("WT", [128, 128], F32, isOutput=False)
    yH = nc.declare_dram_parameter("yH", [NB, 128, M], F32, isOutput=True)

    with tile.TileContext(nc) as tc:
        from contextlib import ExitStack

        with ExitStack() as ctx:
            cp = ctx.enter_context(tc.tile_pool(name="consts", bufs=1))
            pb = ctx.enter_context(tc.tile_pool(name="big", bufs=2))
            psm = ctx.enter_context(tc.tile_pool(name="small", bufs=2))
            pp = ctx.enter_context(
                tc.tile_pool(name="psum", bufs=2, space=bass.MemorySpace.PSUM)
            )

            ones_col = cp.tile([128, 1], F32, tag="ones_col")
            nc.gpsimd.memset(ones_col[:, :], 1.0)
            ones_col_r = cp.tile([128, 2], F32R, tag="ones_col_r")
            nc.vector.tensor_copy(ones_col_r[:, 0:1], ones_col[:, :])
            nc.vector.tensor_copy(ones_col_r[:, 1:2], ones_col[:, :])
            ones_row = cp.tile([1, 128], F32, tag="ones_row")
            nc.gpsimd.memset(ones_row[:, :], 1.0)
            ones_row_r = cp.tile([1, 128], F32R, tag="ones_row_r")
            nc.vector.tensor_copy(ones_row_r[:, :], ones_row[:, :])
            mhalf_f32 = cp.tile([1, 128], F32, tag="mhalf_f32")
            nc.gpsimd.memset(mhalf_f32[:, :], -0.5)
            mhalf_row = cp.tile([1, 128], F32R, tag="mhalf_row")
            nc.vector.tensor_copy(mhalf_row[:, :], mhalf_f32[:, :])
            ident = cp.tile([128, 128], F32, tag="ident")
            make_identity(nc, ident[:, :])
            wt_sb = cp.tile([128, 128], F32, tag="wt")
            nc.sync.dma_start(wt_sb[:, :], WT[:, :])
            wt_r = cp.tile([128, 128], F32R, tag="wt_r")
            nc.vector.tensor_copy(wt_r[:, :], wt_sb[:, :])

            # dense burst of real-shaped matmuls to push the PE HAM clock
            # gate to K=8/8 before the latency-critical phases start
            junk = cp.tile([128, 512], BF16, tag="junk")
            nc.gpsimd.memset(junk[:, :], 0.5)
            warm_ps = pp.tile([128, 512], F32, tag="big", name="warmps")
            for _w in range(24):
                nc.tensor.matmul(
                    warm_ps[:, :], lhsT=junk[:, 0:128], rhs=junk[:, :],
                    start=True, stop=True, skip_group_check=True,
                )

            st = [dict() for _ in range(NB)]

            # ---------- phase: prep (loads, squares, sq row + chunk) ----------
            for b in range(NB):
                s = st[b]
                xt = pb.tile([128, M], F32, tag="xt", bufs=1, name=f"xt{b}")
                nc.sync.dma_start(xt[:, :], xT[b])
                xt_r = pb.tile([128, M], F32R, tag="xt_r", name=f"xtr{b}")
                nc.vector.tensor_copy(xt_r[:, :], xt[:, :])
                xn = pb.tile([128, M], F32, tag="xn", name=f"xn{b}")
                nc.sync.dma_start(xn[:, :], xN[b])
                xsq = pb.tile([128, M], F32R, tag="xsq", bufs=1, name=f"xsq{b}")
                nc.scalar.activation(xsq[:, :], xt_r[:, :], AF.Square)
                sq_ps = pp.tile([1, M], F32, tag="big", name=f"sqps{b}")
                for j in range(NS):
                    nc.tensor.matmul(
                        sq_ps[:, _sl(j)], lhsT=ones_col_r[:, 0:1], rhs=xsq[:, _sl(j)]
                    )
                sq_row = psm.tile([1, M], F32R, tag="sq_row", name=f"sqrow{b}")
                nc.scalar.copy(sq_row[0:1, :], sq_ps[:, :])
                # sq in chunk layout [128, NT] via DMA + PE transpose so the
                # ACT bias sees the same PE-computed sq as the PSUM d2'
                s16 = psm.tile([NT, 128], F32, tag="s16", name=f"s16_{b}")
                nc.sync.dma_start(s16[:, :], sq_row[0:1, :].bitcast(F32))
                sqc_ps = pp.tile([128, NT], F32, tag="big", name=f"sqcps{b}")
                nc.tensor.transpose(sqc_ps[:, :], s16[:, :], ident[0:NT, 0:NT])
                sqc = psm.tile([128, NT], F32, tag="sqc", name=f"sqc{b}")
                nc.scalar.copy(sqc[:, :], sqc_ps[:, :])
                bias_s = psm.tile([128, NT], F32, tag="bias_s", name=f"biass{b}")
                nc.vector.tensor_scalar_add(bias_s[:, :], sqc[:, :], EPS)
                s["xt_r"], s["xn"], s["sq_row"], s["sqc"], s["bias_s"] = (
                    xt_r, xn, sq_row, sqc, bias_s,
                )

            def emit_d2(s, t, d2_ps):
                # PSUM d2'[i, j] = x_i.x_j - sq_j/2   (i in chunk t)
                # full d2 = -2*d2' + sq_i (+EPS), applied via ACT scale/bias
                for j in range(NS):
                    nc.tensor.matmul(
                        d2_ps[:, _sl(j)],
                        lhsT=mhalf_row[0:1, :],
                        rhs=s["sq_row"][0:1, _sl(j)],
                        start=True,
                        stop=False,
                    )
                for j in range(NS):
                    nc.tensor.matmul(
                        d2_ps[:, _sl(j)],
                        lhsT=s["xt_r"][:, _tb(t)],
                        rhs=s["xt_r"][:, _sl(j)],
                        start=False,
                        stop=True,
                    )

            # ---------- phase: pass B (sampled dist sums for h; sqrt set) ----
            # h is a mean over 4.2M off-diag distances; a 4-tile (512-row)
            # sample estimates it to ~4e-5 relative, far below the spacing
            # of adjacent rho values, so ranks are unaffected.
            HS = (0, 4, 8, 12)
            for b in range(NB):
                s = st[b]
                hacc = psm.tile([128, len(HS)], F32, tag="hacc", name=f"hacc{b}")
                for hi, t in enumerate(HS):
                    d2_ps = pp.tile([128, M], F32, tag="big", name=f"d2b{b}_{t}")
                    emit_d2(s, t, d2_ps)
                    scr = pb.tile([128, M], BF16, tag="scr", bufs=1, name=f"sb{b}_{t}")
                    nc.scalar.activation(
                        scr[:, :],
                        d2_ps[:, :],
                        AF.Sqrt,
                        scale=-2.0,
                        bias=s["bias_s"][:, t : t + 1],
                        accum_out=hacc[:, hi : hi + 1],
                    )
                s["hacc"] = hacc

            # ---------- phase: h -> f2 = 1/h^2, exp bias -------------------
            for b in range(NB):
                s = st[b]
                hsum = psm.tile([128, 1], F32, tag="hsum", name=f"hsum{b}")
                nc.vector.tensor_reduce(
                    hsum[:, :], s["hacc"][:, :], axis=AX.X, op=ALU.add
                )
                tot_ps = pp.tile([1, 1], F32, tag="big", name=f"tot{b}")
                nc.tensor.matmul(tot_ps[:, :], lhsT=ones_col[:, :], rhs=hsum[:, :])
                ht = psm.tile([1, 1], F32, tag="ht", name=f"ht{b}")
                nc.vector.tensor_scalar(
                    ht[:, :],
                    tot_ps[:, :],
                    1.0 / (512 * (M - 1)),
                    1e-6,
                    op0=ALU.mult,
                    op1=ALU.max,
                )
                h2 = psm.tile([1, 1], F32, tag="h2", name=f"h2{b}")
                nc.scalar.activation(h2[:, :], ht[:, :], AF.Square)
                rh2 = psm.tile([1, 1], F32, tag="rh2", name=f"rh2{b}")
                nc.vector.reciprocal(rh2[:, :], h2[:, :])
                f_ps = pp.tile([128, 1], F32, tag="big", name=f"fps{b}")
                nc.tensor.matmul(f_ps[:, :], lhsT=ones_row[:, :], rhs=rh2[:, :])
                f_bc = psm.tile([128, 1], F32, tag="fbc", name=f"fbc{b}")
                nc.scalar.copy(f_bc[:, :], f_ps[:, :])
                # exp arg = f2*d2' - f2*sq_i/2  (the dropped EPS is a global
                # factor on every rho -> cannot change any comparison)
                bias_e = psm.tile([128, NT], F32, tag="bias_e", name=f"biase{b}")
                nc.vector.tensor_scalar(
                    bias_e[:, :], s["sqc"][:, :], f_bc[:, 0:1], -0.5,
                    op0=ALU.mult, op1=ALU.mult,
                )
                s["f_bc"], s["bias_e"] = f_bc, bias_e

            # ---------- phase: pass C (rho row-sums; exp/ln table set) --------
            for b in range(NB):
                s = st[b]
                rho = psm.tile([128, NT], F32, tag="rho", name=f"rho{b}")
                for t in range(NT):
                    d2_ps = pp.tile([128, M], F32, tag="big", name=f"d2c{b}_{t}")
                    emit_d2(s, t, d2_ps)
                    scr = pb.tile([128, M], BF16, tag="scr", bufs=1, name=f"sc{b}_{t}")
                    nc.scalar.activation(
                        scr[:, :],
                        d2_ps[:, :],
                        AF.Exp,
                        scale=s["f_bc"][:, :],
                        bias=s["bias_e"][:, t : t + 1],
                        accum_out=rho[:, t : t + 1],
                    )
                s["rho"] = rho

            # ---------- per batch building blocks ----------
            def emit_layout(b):
                """rho as an exact [1, M] row and [128, M] broadcast tile."""
                s = st[b]
                rT_ps = pp.tile([NT, 128], F32, tag="big", name=f"rTps{b}")
                nc.tensor.transpose(rT_ps[:, :], s["rho"][:, :], ident[:, :])
                rT = psm.tile([NT, 128], F32, tag="rT", name=f"rT{b}")
                nc.scalar.copy(rT[:, :], rT_ps[:, :])
                rrow = psm.tile([1, M], F32, tag="rowtmp", bufs=2, name=f"rrow{b}")
                nc.sync.dma_start(rrow[0:1, :], rT[:, :])
                rbc_ps = pp.tile([128, M], F32, tag="big", name=f"rbcps{b}")
                for j in range(NS):
                    nc.tensor.matmul(
                        rbc_ps[:, _sl(j)], lhsT=ones_row[:, :], rhs=rrow[0:1, _sl(j)]
                    )
                rho_bc = pb.tile([128, M], F32, tag="rho_bc", bufs=2, name=f"rbc{b}")
                nc.scalar.copy(rho_bc[:, :], rbc_ps[:, :])
                s["rho_bc"] = rho_bc

            def emit_gstream(b):
                """ranks r_i = #{k: rho_k < rho_i} via compare tiles (DVE +
                GpSimd) column-summed on the PE, then moved to chunk layout."""
                s = st[b]
                r_ps = pp.tile([1, M], F32, tag="big", name=f"rps{b}")
                for t in range(NT):
                    lt = pb.tile([128, M], F32R, tag="lt", bufs=3,
                                 name=f"lg_{b}_{t}")
                    nc.vector.tensor_scalar(
                        lt[:, :], s["rho_bc"][:, :], s["rho"][:, t : t + 1], None,
                        op0=ALU.is_gt,
                    )
                    for j in range(NS):
                        nc.tensor.matmul(
                            r_ps[:, _sl(j)], lhsT=ones_col_r[:, 0:1],
                            rhs=lt[:, _sl(j)],
                            start=(t == 0), stop=(t == NT - 1),
                        )
                r_sb = psm.tile([1, M], F32, tag="rowtmp", bufs=2, name=f"rsb{b}")
                nc.scalar.copy(r_sb[0:1, :], r_ps[:, :])
                r16 = psm.tile([NT, 128], F32, tag="r16", name=f"r16_{b}")
                nc.sync.dma_start(r16[:, :], r_sb[0:1, :])
                rc_ps = pp.tile([128, NT], F32, tag="big", name=f"rcps{b}")
                nc.tensor.transpose(rc_ps[:, :], r16[:, :], ident[0:NT, 0:NT])
                racc = psm.tile([128, NT], F32, tag="racc", name=f"racc{b}")
                nc.scalar.copy(racc[:, :], rc_ps[:, :])
                s["racc"] = racc

            def emit_scalars_uv(b):
                """Dv/dvs/C vectors from ranks; u, v, T; dvs broadcasts."""
                s = st[b]
                racc = s["racc"]
                Dv = psm.tile([128, NT], F32, tag="Dv", name=f"Dv{b}")
                nc.vector.tensor_scalar_add(Dv[:, :], racc[:, :], 1.0)
                lnDv = psm.tile([128, NT], F32, tag="lnDv", name=f"lnDv{b}")
                nc.scalar.activation(lnDv[:, :], Dv[:, :], AF.Ln)
                dvs = psm.tile([128, NT], F32, tag="dvs", name=f"dvs{b}")
                nc.scalar.activation(dvs[:, :], lnDv[:, :], AF.Exp, scale=-0.5)
                g = psm.tile([128, NT], F32, tag="g", name=f"g{b}")
                nc.vector.tensor_scalar(
                    g[:, :], racc[:, :], -1.0, float(M - 1), op0=ALU.mult,
                    op1=ALU.add,
                )
                gm = psm.tile([128, NT], F32, tag="gm", name=f"gm{b}")
                nc.vector.tensor_scalar_max(gm[:, :], g[:, :], 1.0)
                inv = psm.tile([128, NT], F32, tag="inv", name=f"inv{b}")
                nc.vector.reciprocal(inv[:, :], gm[:, :])
                lng = psm.tile([128, NT], F32, tag="lng", name=f"lng{b}")
                nc.scalar.activation(lng[:, :], gm[:, :], AF.Ln)
                inv2 = psm.tile([128, NT], F32, tag="inv2", name=f"inv2{b}")
                nc.scalar.activation(inv2[:, :], inv[:, :], AF.Square)
                c1 = psm.tile([128, NT], F32, tag="c1", name=f"c1{b}")
                nc.vector.tensor_scalar(
                    c1[:, :], lng[:, :], -1.0, H_M - GAMMA, op0=ALU.mult, op1=ALU.add
                )
                c2 = psm.tile([128, NT], F32, tag="c2", name=f"c2{b}")
                nc.vector.scalar_tensor_tensor(
                    c2[:, :], in0=inv[:, :], scalar=-0.5, in1=c1[:, :],
                    op0=ALU.mult, op1=ALU.add,
                )
                Cv = psm.tile([128, NT], F32, tag="Cv", name=f"Cv{b}")
                nc.vector.scalar_tensor_tensor(
                    Cv[:, :], in0=inv2[:, :], scalar=1.0 / 12.0, in1=c2[:, :],
                    op0=ALU.mult, op1=ALU.add,
                )
                dvsC = psm.tile([128, NT], F32, tag="dvsC", name=f"dvsC{b}")
                nc.vector.tensor_mul(dvsC[:, :], dvs[:, :], Cv[:, :])

                u = pb.tile([128, M], F32R, tag="u", bufs=1, name=f"u{b}")
                v = pb.tile([128, M], F32R, tag="v", bufs=1, name=f"v{b}")
                for t in range(NT):
                    nc.vector.tensor_scalar(
                        u[:, _tb(t)], s["xn"][:, _tb(t)], dvs[:, t : t + 1], None,
                        op0=ALU.mult,
                    )
                    nc.vector.tensor_scalar(
                        v[:, _tb(t)], s["xn"][:, _tb(t)], dvsC[:, t : t + 1], None,
                        op0=ALU.mult,
                    )
                s["u"], s["v"] = u, v
                T_ps = pp.tile([128, 2], F32, tag="big", name=f"Tps{b}")
                for t in range(NT):
                    nc.tensor.matmul(
                        T_ps[:, :], lhsT=u[:, _tb(t)], rhs=ones_col_r[:, :],
                        start=(t == 0), stop=(t == NT - 1),
                    )
                T_sb = psm.tile([128, 1], F32, tag="T_sb", name=f"Tsb{b}")
                nc.scalar.copy(T_sb[:, :], T_ps[:, 0:1])
                s["T_sb"] = T_sb

                stk = psm.tile([128, 2 * NT], F32, tag="stk", name=f"stk{b}")
                nc.vector.tensor_copy(stk[:, 0:NT], dvs[:, :])
                nc.vector.tensor_copy(stk[:, NT : 2 * NT], dvsC[:, :])
                stT_ps = pp.tile([2 * NT, 128], F32, tag="big", name=f"stTps{b}")
                nc.tensor.transpose(stT_ps[:, :], stk[:, :], ident[:, :])
                stT = psm.tile([2 * NT, 128], F32R, tag="stT", name=f"stT{b}")
                nc.vector.tensor_copy(stT[:, :], stT_ps[:, :])
                dvs_row = psm.tile([1, M], F32R, tag="rowtmp", bufs=2,
                                   name=f"dr{b}")
                nc.sync.dma_start(dvs_row[0:1, :], stT[0:NT, :])
                dvsC_row = psm.tile([1, M], F32R, tag="rowtmp", bufs=2,
                                    name=f"cr{b}")
                nc.sync.dma_start(dvsC_row[0:1, :], stT[NT : 2 * NT, :])

                dbc_ps = pp.tile([128, M], F32, tag="big", name=f"dbcps{b}")
                for j in range(NS):
                    nc.tensor.matmul(
                        dbc_ps[:, _sl(j)], lhsT=ones_row_r[:, :],
                        rhs=dvs_row[0:1, _sl(j)],
                    )
                dvs_bc = pb.tile([128, M], F32, tag="dvs_bc", bufs=1, name=f"db{b}")
                nc.scalar.copy(dvs_bc[:, :], dbc_ps[:, :])
                cbc_ps = pp.tile([128, M], F32, tag="big", name=f"cbcps{b}")
                for j in range(NS):
                    nc.tensor.matmul(
                        cbc_ps[:, _sl(j)], lhsT=ones_row_r[:, :],
                        rhs=dvsC_row[0:1, _sl(j)],
                    )
                dvsC_bc = pb.tile([128, M], F32, tag="dvsC_bc", bufs=1,
                                  name=f"cb{b}")
                nc.scalar.copy(dvsC_bc[:, :], cbc_ps[:, :])
                s["dvs_bc"], s["dvsC_bc"] = dvs_bc, dvsC_bc

            def emit_l2(b):
                """L pass 2 + P1 = (L@v)^T, P2 = (L@u)^T ([c, i] PSUM)."""
                s = st[b]
                P1_ps = pp.tile([128, M], F32, tag="big", name=f"P1ps{b}")
                P2_ps = pp.tile([128, M], F32, tag="big", name=f"P2ps{b}")
                for t in range(NT):
                    lt = pb.tile([128, M], F32R, tag="lt", bufs=3,
                                 name=f"lt2_{b}_{t}")
                    nc.vector.tensor_scalar(
                        lt[:, :], s["rho_bc"][:, :], s["rho"][:, t : t + 1], None,
                        op0=ALU.is_gt,
                    )
                    for j in range(NS):
                        nc.tensor.matmul(
                            P2_ps[:, _sl(j)], lhsT=s["u"][:, _tb(t)],
                            rhs=lt[:, _sl(j)],
                            start=(t == 0), stop=(t == NT - 1),
                        )
                    for j in range(NS):
                        nc.tensor.matmul(
                            P1_ps[:, _sl(j)], lhsT=s["v"][:, _tb(t)],
                            rhs=lt[:, _sl(j)],
                            start=(t == 0), stop=(t == NT - 1),
                        )
                s["P1_ps"], s["P2_ps"] = P1_ps, P2_ps

            def emit_z(b):
                """z = dvs_i*P1 - (dvs*C)_i*(P2 - T_c)   ([c, i] layout)."""
                s = st[b]
                zt1 = pb.tile([128, M], F32, tag="v", bufs=1, name=f"zt1{b}")
                nc.vector.scalar_tensor_tensor(
                    zt1[:, :], in0=s["P2_ps"][:, :], scalar=s["T_sb"][:, 0:1],
                    in1=s["dvsC_bc"][:, :], op0=ALU.subtract, op1=ALU.mult,
                )
                zt2 = pb.tile([128, M], F32, tag="u", bufs=1, name=f"zt2{b}")
                nc.vector.scalar_tensor_tensor(
                    zt2[:, :], in0=s["P1_ps"][:, :], scalar=0.0,
                    in1=s["dvs_bc"][:, :], op0=ALU.bypass, op1=ALU.mult,
                )
                z = pb.tile([128, M], F32R, tag="z", name=f"z{b}")
                nc.vector.tensor_sub(z[:, :], zt2[:, :], zt1[:, :])
                s["z"] = z

            def emit_proj(b):
                """yT = (W @ z) with constant stationary; SiLU; store [o, i]."""
                s = st[b]
                yT_ps = pp.tile([128, M], F32, tag="big", name=f"yTps{b}")
                for j in range(NS):
                    nc.tensor.matmul(
                        yT_ps[:, _sl(j)], lhsT=wt_r[:, :], rhs=s["z"][:, _sl(j)]
                    )
                sg = pb.tile([128, M], F32, tag="sg", bufs=1, name=f"sgp{b}")
                nc.scalar.activation(sg[:, :], yT_ps[:, :], AF.Sigmoid)
                y_sb = pb.tile([128, M], F32, tag="y_sb", bufs=1, name=f"ysb{b}")
                nc.vector.tensor_mul(y_sb[:, :], yT_ps[:, :], sg[:, :])
                nc.sync.dma_start(yH[b], y_sb[:, :])

            # ---------- schedule: interleave batches for engine overlap ------
            emit_layout(0)
            emit_layout(1)
            emit_gstream(0)
            emit_scalars_uv(0)
            emit_l2(0)
            emit_z(0)
            emit_gstream(1)
            emit_scalars_uv(1)
            emit_l2(1)
            emit_z(1)
            emit_proj(0)
            emit_proj(1)

    nc.compile()
    return nc


_CACHED_NC = None


def _get_nc():
    global _CACHED_NC
    if _CACHED_NC is None:
        _CACHED_NC = build_kernel()
    return _CACHED_NC


def make_in_maps(x, W):
    x = np.asarray(x, dtype=np.float32)
    W = np.asarray(W, dtype=np.float32)
    wt = np.ascontiguousarray(W.T)
    in_maps = []
    for core in range(N_CORES):
        xb = x[core * NB : (core + 1) * NB]                       # [NB, M, C]
        xt = np.ascontiguousarray(xb.transpose(0, 2, 1))          # [NB, C, M]
        # xn[b, p, t*128+c] = x[b, t*128+p, c]
        xn = np.ascontiguousarray(
            xb.reshape(NB, NT, 128, C).transpose(0, 2, 1, 3).reshape(NB, 128, M)
        )
        in_maps.append({"xT": xt, "xN": xn, "WT": wt})
    return in_maps


def unshard_output(results):
    outs = []
    for core in range(N_CORES):
        yh = results[core]["yH"]                                  # [NB, C, M]
        outs.append(yh.transpose(0, 2, 1))                        # [NB, M, C]
    return np.concatenate(outs, axis=0).astype(np.float32)


def run(x, W, trace=False, trace_kwargs=None):
    nc = _get_nc()
    res = run_bass_kernel_spmd(
        nc,
        make_in_maps(x, W),
        list(range(N_CORES)),
        trace=trace,
        **(trace_kwargs or {}),
    )
    return unshard_output(res.results), res


def kernel(x, W):
    y, _ = run(x, W, trace=False)
    return y
